# revision 25
# baseline (speedup 1.0000x reference)
"""Trainium2 Bass kernel for nn_Attn_time (sparse time-similarity attention).

reference:
    energies[i, j] = time_sim_mat[cur[i], his[j]]   # [4096, 8192]
    out = softmax(energies, axis=-1)

Structure exploited: cur/his index into only T=1024 time buckets, so
    out[i, j] = S[cur[i], j]  where  S = softmax_rows(time_sim_mat[:, his])
and S is only [1024, 8192]. Column-shard S across the 8 cores (1024 j each):

 - G[t, j] = sum_u M[t, u] * H[u, j] with one-hot H[u, j] = (his[j] == u),
   run on the TensorEngine in fp16 (one-hot selection is exact; fp16
   rounds energies to ~2e-3 absolute, i.e. ~0.2% on exp).
 - softmax denominator, fully local (no collectives):
       rowsum[t] = sum_u exp(M[t, u]) * cnt[u]
                 = sum_u exp(M[t, u] + ln cnt[u]),   cnt = bincount(his)
   computed as a DVE add + ScalarE exp with accum_out.
 - S rows (bf16) are parked in DRAM and row-gathered by `cur` with
   dma_gather (4096 row descriptors of 2 KiB); the output store casts
   bf16->f32 inside the SWDGE DMA.
Per-core output shard: out[:, k*1024:(k+1)*1024]; host concatenates.
"""

import numpy as np

import concourse.bass as bass
import concourse.tile as tile
from concourse import bacc, mybir
from concourse.bass_utils import run_bass_kernel_spmd
from bass_rust import add_dep_helper

T = 1024          # time buckets
SEQ = 8192        # len(his)
STATE = 4096      # len(cur)
NCORES = 8
JSH = SEQ // NCORES        # j columns per core = 1024
GCHUNK = 512               # gather rows per dma_gather chunk
NCHUNKS = STATE // GCHUNK  # 4

F32 = mybir.dt.float32
F16 = mybir.dt.float16
BF16 = mybir.dt.bfloat16
I16 = mybir.dt.int16


def build_kernel():
    nc = bacc.Bacc("TRN2", target_bir_lowering=False, debug=False,
                   num_devices=NCORES, num_swdge_queues=2,
                   dynamic_dma_scratch_size=32768)

    m_param = nc.dram_tensor("m16", [T, T], F16, kind="ExternalInput")
    his_param = nc.dram_tensor("his_f32", [JSH], F32, kind="ExternalInput")
    cur_param = nc.dram_tensor("cur_idx16", [128, STATE // 16], I16,
                               kind="ExternalInput")
    ucol_param = nc.dram_tensor("ucol", [128, 8], F32, kind="ExternalInput")
    lncnt_param = nc.dram_tensor("lncnt", [T], F16, kind="ExternalInput")
    out_param = nc.dram_tensor("out", [STATE, JSH], F32, kind="ExternalOutput")

    with tile.TileContext(nc, num_cores=NCORES) as tc:
        with (
            tc.tile_pool(name="singles", bufs=1) as singles,
            tc.tile_pool(name="rstmp", bufs=2) as rstmp,
            tc.tile_pool(name="gat", bufs=1) as gat,
            tc.tile_pool(name="psum", bufs=2, space="PSUM") as psum,
            tc.tile_pool(name="dram", bufs=1, space="DRAM") as dram,
        ):
            # ---- persistent SBUF tiles
            mt_sb = singles.tile([128, 8, T], F16)       # M^T [u%128, u//128, t]
            m_tu = singles.tile([128, 8, T], F16)        # M   [t%128, t//128, u]
            h_sb = singles.tile([128, 8, JSH], F16)      # one-hot his shard
            eg_sb = singles.tile([128, 8, JSH], BF16)    # exp(G) -> S, t=m*128+p
            his_sb = singles.tile([128, JSH], F32)       # his bcast to all parts
            lncnt_sb = singles.tile([128, T], F16)       # ln cnt bcast
            idx_sb = singles.tile([128, STATE // 16], I16)
            ucol_sb = singles.tile([128, 8], F32)        # ucol[p,c] = c*128+p
            rs_sb = singles.tile([128, 8], F32)          # rowsum, t=b*128+p
            inv_sb = singles.tile([128, 8], F32)         # 1/rowsum

            s_dram = dram.tile([T, JSH], BF16)

            # ---- small input loads (sync ring)
            nc.sync.dma_start(out=idx_sb, in_=cur_param.ap())
            nc.sync.dma_start(out=ucol_sb, in_=ucol_param.ap())
            nc.sync.dma_start(
                out=his_sb,
                in_=bass.AP(tensor=his_param, offset=0, ap=[[0, 128], [1, JSH]]),
            )

            # ---- M^T via xbar DMA transpose (fp16), feeds PE - keep first
            for c in range(8):
                nc.scalar.dma_start_transpose(
                    out=mt_sb[:, c, :],
                    in_=m_param.ap()[:, c * 128:(c + 1) * 128],
                )

            # ---- one-hot H[u, j] = (his[j] == u), u = c*128+p
            for c in range(8):
                nc.vector.tensor_scalar(
                    out=h_sb[:, c, :],
                    in0=his_sb,
                    scalar1=ucol_sb[:, c:c + 1],
                    scalar2=None,
                    op0=mybir.AluOpType.is_equal,
                )

            # ---- local softmax denominator (loads on the scalar HWDGE ring):
            # rowsum[t] = sum_u exp(M[t,u] + ln cnt[u]) via ACT accum_out
            nc.sync.dma_start(
                out=lncnt_sb,
                in_=bass.AP(tensor=lncnt_param, offset=0, ap=[[0, 128], [1, T]]),
            )
            nc.sync.dma_start(
                out=m_tu,
                in_=m_param.ap().rearrange("(b p) u -> p b u", p=128),
            )
            for b in range(8):
                xs = rstmp.tile([128, T], F32, tag="xs")
                es = rstmp.tile([128, T], BF16, tag="es")
                nc.vector.tensor_add(xs, m_tu[:, b, :], lncnt_sb)
                nc.scalar.activation(
                    out=es,
                    in_=xs,
                    func=mybir.ActivationFunctionType.Exp,
                    accum_out=rs_sb[:, b:b + 1],
                )
            nc.vector.reciprocal(out=inv_sb, in_=rs_sb)

            # ---- gather descriptors pre-generate on SWDGE queue 1 (their
            # only sync dep is idx_sb, so desc-gen overlaps the matmul
            # phase; the RAW dep on s_dram defers to the trigger below).
            gat_sem = nc.alloc_semaphore("gat_dma")
            gtiles = []
            for ch in range(NCHUNKS):
                g = gat.tile([128, GCHUNK // 128, JSH], BF16, name=f"g{ch}",
                             tag=f"g{ch}")
                w = GCHUNK // 16
                nc.gpsimd.dma_gather(
                    g,
                    s_dram[:],
                    idx_sb[:, ch * w:(ch + 1) * w],
                    num_idxs=GCHUNK,
                    num_idxs_reg=GCHUNK,
                    elem_size=JSH,
                    elem_step=JSH,
                    prepare_only=True,
                    sem=gat_sem,
                    queue_num=1,
                )
                gtiles.append(g)

            swrites = []
            # ---- G = M @ H on PE (fp16, f32 accum), exp, scale, park
            for m in range(8):          # t block (psum partition = t%128)
                for n in range(2):      # j half (512 wide)
                    pg = psum.tile([128, 512], F32)
                    for c in range(8):  # contraction over u
                        nc.tensor.matmul(
                            pg,
                            mt_sb[:, c, m * 128:(m + 1) * 128],
                            h_sb[:, c, n * 512:(n + 1) * 512],
                            start=(c == 0),
                            stop=(c == 7),
                        )
                    nc.scalar.activation(
                        out=eg_sb[:, m, n * 512:(n + 1) * 512],
                        in_=pg,
                        func=mybir.ActivationFunctionType.Exp,
                    )
                nc.vector.tensor_scalar_mul(
                    eg_sb[:, m, :], eg_sb[:, m, :], inv_sb[:, m:m + 1]
                )
                swrites.append(nc.sync.dma_start(
                    out=s_dram[m * 128:(m + 1) * 128, :],
                    in_=eg_sb[:, m, :],
                ).ins)

            # ---- fire the pre-generated gathers, then store with cast.
            # The preps were traced before the S writes, so their RAW dep on
            # s_dram is absent; the trigger's wait assignment is prep-only.
            # Order the fire after the S writes via a tiny Pool-engine read
            # of s_dram: its RAW wait stalls the Pool stream until S landed,
            # and the trigger issues right after it.
            sbar = gat.tile([128, 8], BF16, name="sbar", tag="sbar")
            sbar_dma = nc.gpsimd.dma_start(
                out=sbar,
                in_=s_dram[:].rearrange("(m p) j -> p m j", p=128)[:, :, 0],
            )
            trig = nc.gpsimd.trigger_dma(count=None, queue_num=1)
            add_dep_helper(trig.ins, sbar_dma.ins, False,
                           "fire gathers only after S landed in DRAM")
            for ch in range(NCHUNKS):
                out_view = out_param.ap()[ch * GCHUNK:(ch + 1) * GCHUNK, :]
                st = nc.gpsimd.dma_start(
                    out=out_view.rearrange("(q p) j -> p q j", p=128),
                    in_=gtiles[ch],
                )
                add_dep_helper(st.ins, trig.ins, False,
                               "store issues after the gathers fired")

    nc.compile()
    return nc


_NC_CACHE = None
_last_in_maps = None


def _get_nc():
    global _NC_CACHE
    if _NC_CACHE is None:
        _NC_CACHE = build_kernel()
    return _NC_CACHE


def kernel(his, cur, time_sim_mat):
    his = np.asarray(his)
    cur = np.asarray(cur)
    m = np.asarray(time_sim_mat, dtype=np.float32)

    m16 = np.ascontiguousarray(m.astype(np.float16))

    # cur indices, wrapped for dma_gather: chunk ch uses idx columns
    # [ch*64, (ch+1)*64); index g of a chunk sits at [g%16, g//16].
    a = np.zeros((16, STATE // 16), dtype=np.int16)
    w = GCHUNK // 16
    for ch in range(NCHUNKS):
        blk = cur[ch * GCHUNK:(ch + 1) * GCHUNK].astype(np.int16)
        a[:, ch * w:(ch + 1) * w] = blk.reshape(w, 16).T
    cur16 = np.tile(a, (8, 1))  # replicate across the 8 gpsimd core groups

    p = np.arange(128, dtype=np.float32)
    ucol = np.ascontiguousarray(
        p[:, None] + 128.0 * np.arange(8, dtype=np.float32)[None, :])

    # ln(bincount(his)); empty buckets get a large negative (exp -> 0)
    cnt = np.bincount(np.asarray(his, dtype=np.int64), minlength=T).astype(
        np.float32)
    with np.errstate(divide="ignore"):
        lncnt = np.where(cnt > 0, np.log(cnt), -1e4).astype(np.float32)
    lncnt16 = np.ascontiguousarray(lncnt.astype(np.float16))

    in_maps = []
    for k in range(NCORES):
        in_maps.append({
            "m16": m16,
            "his_f32": np.ascontiguousarray(
                his[k * JSH:(k + 1) * JSH].astype(np.float32)),
            "cur_idx16": cur16,
            "ucol": ucol,
            "lncnt": lncnt16,
        })

    global _last_in_maps
    _last_in_maps = in_maps

    nc = _get_nc()
    res = run_bass_kernel_spmd(nc, in_maps, core_ids=list(range(NCORES)))
    out = np.concatenate([res.results[k]["out"] for k in range(NCORES)], axis=1)
    return out
